# revision 57
# baseline (speedup 1.0000x reference)
"""Trainium2 8-core SPMD kernel for MQA attention with relative position bias.

Reference computation (b=2, n=2048, D=1024, h=8, dh=64, MQA single k/v head):
    q  = x @ Wq;  kv = x @ Wkv;  k, v = kv[..., :64], kv[..., 64:]
    sim = (q[b,h,i,:] . k[b,j,:]) * dh**-0.5 + rel_pos_bias[h,i,j]   (causal masked)
    out = softmax(sim) @ v  -> reshape -> @ Wo + bo

v5 design (collective-free, 1 batch per core, exact causal, per-core
programs):
  - 4 cores per batch. Core c (b=c//4, cp=c%4) owns q-tiles
    {cp, 7-cp, 8+cp, 15-cp} of its batch (sorted ascending = slots 0..3)
    with EXACT causal extents (34 (slot, j-tile) pairs on every core --
    perfectly balanced, zero padding). Only 4 distinct programs exist
    (one per cp); the two batch halves share them. All 8 NEFFs dispatch
    concurrently via PJRT (async; no collectives, no rendezvous).
  - k/v computed only for the core's own batch (4 512-token windows).
  - All big inputs are pre-arranged on the host into the exact SBUF
    [128, free] layouts so every DMA is a flat 2D contiguous copy
    (n-D rearrange APs cost 3-6us of HWDGE descriptor issue each).
  - Scores transposed (keys on partitions), 4 heads per N=512 matmul,
    f32 PSUM; a global PV-deferral queue (depth 3) crosses slot
    boundaries so the PE never drains at slot ends. Bare keep-warm
    LDWEIGHTS per pair hold HAM's activity window busy (cold K=4/8
    halves the PE clock; idle >~a window re-throttles).
  - rel-pos bias applied multiplicatively AFTER the exp: exp(b) built
    on host (bf16, causal-masked entries exactly 0); exp(qk/8) on
    ScalarE; the bias multiply on DVE (GPSIMD while a tail drains --
    DVE saturates in those windows).
  - Softmax denominator: the PV stationary is [v | 64 ones-columns], so
    PSUM partitions 64:127 hold the denominator replicated -- the
    partition broadcast is free on the PE array. 1/l via
    reciprocal_approx_fast (5x faster than InstReciprocal; staged
    through SBUF -- its bit-trick seed misreads PSUM).
  - Slot tails are 4 tasks drained one per pair (normalize halves, then
    out-proj head-pairs); out-proj rides the freed pv-ring PSUM slot,
    bo is added during the PSUM->bf16 cast (tensor_add against a
    GPSIMD-broadcast bo), out DMA (bf16) on the ACT HWDGE queue.
"""

import os
import sys

import numpy as np

sys.path.insert(0, "/opt/trn_rl_repo")

import ml_dtypes

BF16 = ml_dtypes.bfloat16

# ---- problem constants (hardcoded per the harness contract) ----
B = 2
N = 2048
DIM = 1024
HEADS = 8
DH = 64
INNER = HEADS * DH  # 512
P = 128
NT = N // P  # 16 q/k tiles per batch
EXTS = (4, 8, 12, 16)  # program extents per slot (ascending)
NPAIR = sum(EXTS)  # 40
NSLOT = 4
NCORES = 8
TOK_OWN = NSLOT * P  # 512 own tokens per core
NWIN = N // 512  # 4 kv windows (own batch only)

_CACHE = {}


def _q_tiles(c):
    cp = c % 4
    return sorted([cp, 7 - cp, 8 + cp, 15 - cp])


def build_graph(
    cp,
    warmup=10,
    st_bufs=2,
    pv_bufs=2,
    pv_depth=3,
    gps_every=1000,
):
    """Per-core-position program: cp = core % 4. Slots own the sorted
    q-tiles {cp, 7-cp, 8+cp, 15-cp} with EXACT causal extents -- no padded
    pairs. Only 4 distinct programs exist (the batch halves share them)."""
    exts = tuple(t + 1 for t in _q_tiles(cp))  # e.g. cp=0 -> (1, 8, 9, 16)
    npair = sum(exts)  # 34 for every cp
    nbias_groups = [(e + 3) // 4 for e in exts]
    import concourse.bass as bass
    import concourse.bacc as bacc
    import concourse.mybir as mybir
    import concourse.tile as tile

    dt = mybir.dt
    f32, f32r, bf16 = dt.float32, dt.float32r, dt.bfloat16
    f16 = dt.float16
    AF = mybir.ActivationFunctionType

    nc = bacc.Bacc(None, target_bir_lowering=False)

    # ---- I/O ----
    # all big inputs are pre-arranged on the host into the exact SBUF
    # [128-partition, free] layout -- every DMA is a plain 2D contiguous
    # copy (3D rearrange APs cost 3-6us of descriptor issue each)
    xTb_t = nc.dram_tensor("xTb", [P, NWIN * 8 * 512], bf16, kind="ExternalInput")
    xTo_t = nc.dram_tensor("xTo", [P, 8 * TOK_OWN], bf16, kind="ExternalInput")
    Wq_t = nc.dram_tensor("Wq", [P, 8 * INNER], bf16, kind="ExternalInput")
    Wkv_t = nc.dram_tensor("Wkv", [P, 8 * 2 * DH], bf16, kind="ExternalInput")
    Wo_t = nc.dram_tensor("Wo", [P, 4 * DIM], bf16, kind="ExternalInput")
    bo_t = nc.dram_tensor("bo", [1, DIM], f32, kind="ExternalInput")
    ident_t = nc.dram_tensor("ident", [P, DH], bf16, kind="ExternalInput")
    ones_r_t = nc.dram_tensor("ones_r", [1, P], f32r, kind="ExternalInput")
    # biasT[pair, j, h, q]: transposed, causal-masked, exp'd bias
    bias_t = nc.dram_tensor("biasT", [P, npair * HEADS * P], bf16, kind="ExternalInput")
    out_t = nc.dram_tensor("out", [TOK_OWN, DIM], bf16, kind="ExternalOutput")

    with tile.TileContext(nc) as tc:
        with (
            tc.tile_pool(name="const", bufs=1) as cpool,
            tc.tile_pool(name="bias", bufs=5) as bpool,
            tc.tile_pool(name="pt", bufs=4) as ptpool,
            tc.tile_pool(name="at", bufs=3) as atpool,
            tc.tile_pool(name="ob", bufs=3) as obpool,
            tc.tile_pool(name="ps", bufs=1, space="PSUM") as pspool,
        ):
            # ---- DMA queue split: sync carries the x windows + bias
            # stream; the weights + small consts ride the ACT queue so the
            # two queues' transfers overlap in the prologue (outputs join
            # the ACT queue later, after the prologue drains).
            xTb_sb = cpool.tile([P, 8 * N], bf16, tag="xTb_sb")
            xTo_sb = cpool.tile([P, 8 * TOK_OWN], bf16, tag="xTo_sb")
            Wq_sb = cpool.tile([P, 8 * INNER], bf16, tag="Wq_sb")
            Wo_sb = cpool.tile([P, 4 * DIM], bf16, tag="Wo_sb")

            def xtb_load(w):
                ws = slice(w * 8 * 512, (w + 1) * 8 * 512)
                nc.sync.dma_start(out=xTb_sb[:, ws], in_=xTb_t[:, ws])

            bias_tiles = {}

            def get_bias(sl, j4):
                """Bias tile for pairs j4*4 .. min(j4*4+4, ext) of slot sl."""
                key = (sl, j4)
                if key in bias_tiles:
                    return bias_tiles[key]
                npr = min(4, exts[sl] - j4 * 4)
                pair = sum(exts[:sl]) + j4 * 4
                t = bpool.tile(
                    [P, 4 * HEADS * P], bf16, tag="bias", name=f"bias{sl}_{j4}"
                )
                nc.sync.dma_start(
                    out=t[:, 0 : npr * HEADS * P],
                    in_=bias_t[:, pair * HEADS * P : (pair + npr) * HEADS * P],
                )
                bias_tiles[key] = t
                return t

            # sync queue: kv window 0 first, then bias/x stream;
            # ACT queue (in parallel): Wkv, xTo, Wq, then the small consts
            xtb_load(0)  # kv window 0
            Wkv_sb = cpool.tile([P, 8 * 2 * DH], bf16, tag="Wkv_sb")
            nc.sync.dma_start(out=Wkv_sb[:], in_=Wkv_t[:])
            nc.sync.dma_start(out=xTo_sb[:], in_=xTo_t[:])
            nc.sync.dma_start(out=Wq_sb[:], in_=Wq_t[:])
            pre_groups = [(sl, j4) for sl in range(NSLOT)
                          for j4 in range(nbias_groups[sl])]
            get_bias(*pre_groups[0])
            xtb_load(1)  # kv window 1
            get_bias(*pre_groups[1])
            nc.sync.dma_start(out=Wo_sb[:], in_=Wo_t[:])
            ident_sb = cpool.tile([P, DH], bf16, tag="ident_sb")
            nc.sync.dma_start(out=ident_sb[:], in_=ident_t[:])
            ones128 = cpool.tile([1, P], f32r, tag="ones128")
            nc.sync.dma_start(out=ones128[:], in_=ones_r_t[:])
            bo_sb = cpool.tile([1, DIM], f32, tag="bo_sb")
            nc.sync.dma_start(out=bo_sb[:], in_=bo_t[:])
            xtb_load(2)  # kv window 2
            get_bias(*pre_groups[2])
            xtb_load(3)  # kv window 3
            get_bias(*pre_groups[3])
            # bias-add operand for the output tail: bo broadcast once to all
            # 128 partitions (GPSIMD is otherwise idle in the prologue)
            bo_bc = cpool.tile([P, DIM], f32, tag="bo_bc")
            nc.gpsimd.partition_broadcast(bo_bc[:, :], bo_sb[:, :])

            # ---- k/v projection over own batch, window by window ----
            kvT_sb = cpool.tile([P, N], bf16, tag="kvT_sb")
            kT2 = kvT_sb[0:DH, :]
            vTs = kvT_sb[DH:P, :]
            # PV stationary per j-tile: [128 j, 128] = [v (64 cols) | ones
            # (64 cols)] -> PV out partitions 64:127 all hold the softmax
            # denominator (the partition-broadcast comes free from the PE)
            VST = cpool.tile([P, NT * P], bf16, tag="VST")
            nc.gpsimd.memset(VST[:, :], 1.0)

            def emit_kv(w):
                kvps = pspool.tile([P, 512], f32, tag="sT", name=f"kv{w}", bufs=st_bufs)
                for fc in range(8):
                    nc.tensor.matmul(
                        kvps[:, :],
                        Wkv_sb[:, fc * 2 * DH : (fc + 1) * 2 * DH],
                        xTb_sb[:, (w * 8 + fc) * 512 : (w * 8 + fc + 1) * 512],
                        start=(fc == 0),
                        stop=(fc == 7),
                    )
                nc.vector.tensor_copy(kvT_sb[:, w * 512 : (w + 1) * 512], kvps[:, :])

            def emit_tp(w):
                tp = pspool.tile([P, 4 * DH], bf16, tag="sT", name=f"tp{w}", bufs=st_bufs)
                for t4 in range(4):
                    t = w * 4 + t4
                    nc.tensor.matmul(
                        tp[:, t4 * DH : (t4 + 1) * DH],
                        vTs[:, t * P : (t + 1) * P],
                        ident_sb[DH:P, :],
                        is_transpose=True,
                        start=True,
                        stop=True,
                        skip_group_check=True,
                    )
                nc.vector.tensor_copy(
                    VST[:, w * 4 * P : (w * 4 + 4) * P]
                    .rearrange("p (t d) -> p t d", t=4)[:, :, 0:DH],
                    tp[:, :].rearrange("p (t d) -> p t d", t=4),
                )

            # ---- PE warmup: ramp the clock while the first DMAs land ----
            # VST is GPSIMD-memset (no DMA dependency) at ~6.5us -- the PE
            # warms from there until kv window 0 arrives
            wps = pspool.tile([P, 512], f32, tag="pv", name="wps", bufs=pv_bufs)
            for i in range(warmup):
                nc.tensor.matmul(
                    wps[:, :], VST[:, 0:P], VST[:, 0:512],
                    start=True, stop=True, skip_group_check=True,
                )
            # Exp-table preload reads the memset VST too (no DMA wait)
            scr = cpool.tile([1, 4], f32, tag="scr")
            nc.scalar.activation(scr[:, :], VST[0:1, 0:4], AF.Exp)

            # ---- q projection (own tokens) ----
            qT_sb = cpool.tile([DH, HEADS * TOK_OWN], bf16, tag="qT_sb")

            def emit_q(hp):
                qps = pspool.tile([P, TOK_OWN], f32, tag="pv", name=f"qps{hp}", bufs=pv_bufs)
                for fc in range(8):
                    nc.tensor.matmul(
                        qps[:, :],
                        Wq_sb[:, fc * INNER + hp * P : fc * INNER + (hp + 1) * P],
                        xTo_sb[:, fc * TOK_OWN : (fc + 1) * TOK_OWN],
                        start=(fc == 0),
                        stop=(fc == 7),
                    )
                # PSUM->SBUF casts on ScalarE (no exps yet in the prologue;
                # DVE needs its headroom for the stream muls)
                nc.scalar.copy(
                    qT_sb[0:DH, (2 * hp) * TOK_OWN : (2 * hp + 1) * TOK_OWN],
                    qps[0:DH, :],
                )
                nc.scalar.copy(
                    qT_sb[0:DH, (2 * hp + 1) * TOK_OWN : (2 * hp + 2) * TOK_OWN],
                    qps[DH:P, :],
                )

            emit_kv(0)
            emit_q(0)
            emit_q(1)
            emit_q(2)
            emit_q(3)
            emit_tp(0)
            qT3 = qT_sb[0:DH, :].rearrange("p (h t) -> p h t", h=HEADS)

            # ---- attention + output projection, software-pipelined ----
            # Flat (slot, jt) pair stream with a GLOBAL PV deferral queue:
            # PV emission trails scores by pv_depth pairs ACROSS slot
            # boundaries, so the PE never drains at a slot end waiting on
            # the exp->mul chain of the slot's last pairs. A slot's tail
            # (normalize + out-proj) is emitted right after its final PV,
            # which by then sits ~pv_depth pairs into the next slot.
            flat_pairs = [(sl, jt) for sl in range(NSLOT)
                          for jt in range(exts[sl])]
            splice_kv, splice_tp = {}, {}
            for w in (1, 2, 3):
                first = next(
                    (i for i, (sl, jt) in enumerate(flat_pairs) if jt >= 4 * w),
                    len(flat_pairs) - 2,
                )
                splice_kv[flat_pairs[max(1, first - 4)]] = w
                splice_tp[flat_pairs[max(2, first - 2)]] = w

            # ---- slot tails, half-pipelined ----
            # Normalize: attnT = pv[0:64] * (1/l). The denominator sits
            # replicated on pv partitions 64:127 (ones-columns in VST), so
            # the reciprocal runs 64-partition-parallel on DVE -- no
            # partition broadcast, no single-partition 6.5us reciprocal.
            # Each tail is 4 tasks drained one-per-pair so the PE stream
            # never queues behind the ACT/DVE normalize chain:
            #   A0/A1: per column-half: stage denom in SBUF (ScalarE; the
            #          approx_fast bit-trick misreads PSUM), fast-recip,
            #          normalize top rows + the head-shifted duplicate
            #   B0/B1: out-proj head-pairs 0-1 then 2-3 (+casts, out DMA)
            attn_tiles = {}

            def tail_a(sl, pv, half):
                hc = slice(half * 4 * P, (half + 1) * 4 * P)
                l_sb = ptpool.tile([DH, 4 * P], f32, tag="pe", name=f"l{sl}_{half}")
                if sl == NSLOT - 1:
                    # final tail: the exp stream is drained, ACT is idle --
                    # staging the denominator there overlaps DVE's last
                    # pair-muls (mid-stream tails must NOT do this: ACT's
                    # strict FIFO would head-of-line-block the exps)
                    nc.scalar.copy(l_sb[:, :], pv[DH:P, hc])
                else:
                    nc.vector.tensor_copy(l_sb[:, :], pv[DH:P, hc])
                recip = ptpool.tile([DH, 4 * P], f32, tag="pe", name=f"rc{sl}_{half}")
                nc.vector.reciprocal_approx_fast(out=recip[:, :], in_=l_sb[:, :])
                if half == 0:
                    attnT = atpool.tile(
                        [P, HEADS * P], bf16, tag="at", name=f"at{sl}"
                    )
                    attn_tiles[sl] = attnT
                else:
                    attnT = attn_tiles[sl]
                nc.vector.tensor_mul(attnT[0:DH, hc], pv[0:DH, hc], recip[:, :])
                if half == 0:
                    # shifted dup heads 1..3 (cols 0:3P <- pv/recip cols P:4P)
                    nc.vector.tensor_mul(
                        attnT[DH:P, 0 : 3 * P], pv[0:DH, P : 4 * P],
                        recip[:, P : 4 * P],
                    )
                else:
                    # shifted dup heads 4..7 (cols 3P:7P <- pv/recip 4P:8P)
                    nc.vector.tensor_mul(
                        attnT[DH:P, 3 * P : 7 * P], pv[0:DH, 4 * P : HEADS * P],
                        recip[:, :],
                    )

            ops_tiles = {}

            def tail_b(sl, part):
                attnT = attn_tiles[sl]
                if part == 0:
                    ops = pspool.tile(
                        [P, DIM], f32, tag="pv", name=f"op{sl}", bufs=pv_bufs
                    )
                    ops_tiles[sl] = ops
                else:
                    ops = ops_tiles[sl]
                for fs_h in range(2):
                    fs = slice(fs_h * 512, (fs_h + 1) * 512)
                    for hp in (2 * part, 2 * part + 1):
                        nc.tensor.matmul(
                            ops[:, fs],
                            attnT[:, 2 * hp * P : (2 * hp + 1) * P],
                            Wo_sb[:, hp * DIM + fs_h * 512 : hp * DIM + (fs_h + 1) * 512],
                            start=(hp == 0),
                            stop=(hp == 3),
                            skip_group_check=True,
                        )
                if part == 1:
                    orow = sl * P
                    ob_sb = obpool.tile([P, DIM], bf16, tag="ob", name=f"ob{sl}")
                    for fs_h in range(2):
                        fs = slice(fs_h * 512, (fs_h + 1) * 512)
                        nc.vector.tensor_add(ob_sb[:, fs], ops[:, fs], bo_bc[:, fs])
                        nc.scalar.dma_start(
                            out=out_t[orow : orow + P, fs], in_=ob_sb[:, fs]
                        )

            pvs = {}
            pend_pv = []  # (sl, jt, pt_sb) with PV not yet emitted
            tail_tasks = []  # closures, drained one per pair

            def emit_pv(sl, jt, pt_sb):
                ext = exts[sl]
                pv = pvs[sl]
                g = jt * P
                for half in range(2):
                    nc.tensor.matmul(
                        pv[:, half * 512 : (half + 1) * 512],
                        VST[:, g : g + P],
                        pt_sb[:, half * 512 : (half + 1) * 512],
                        start=(jt == 0),
                        stop=(jt == ext - 1),
                        skip_group_check=True,
                    )
                if jt == ext - 1:
                    tail_a(sl, pv, 0)
                    tail_tasks.extend(
                        [
                            lambda s=sl, p=pv: tail_a(s, p, 1),
                            lambda s=sl: tail_b(s, 0),
                            lambda s=sl: tail_b(s, 1),
                        ]
                    )

            mul_i = 0
            gps_cool = 0
            for sl, jt in flat_pairs:
                if jt == 0:
                    pvs[sl] = pspool.tile(
                        [P, HEADS * P], f32, tag="pv", name=f"pv{sl}",
                        bufs=pv_bufs,
                    )
                if jt % 4 == 0:
                    bias_sb = get_bias(sl, jt // 4)
                    # prefetch the next 4-pair group's tile
                    if jt + 4 < exts[sl]:
                        get_bias(sl, jt // 4 + 1)
                    elif sl + 1 < NSLOT:
                        get_bias(sl + 1, 0)
                bcol = (jt % 4) * HEADS * P
                sT = pspool.tile(
                    [P, HEADS * P], f32, tag="sT", name=f"sT{sl}_{jt}",
                    bufs=st_bufs,
                )
                kcol = jt * P
                for half in range(2):
                    nc.tensor.matmul(
                        sT[:, half * 512 : (half + 1) * 512],
                        kT2[:, kcol : kcol + P],
                        qT3[:, 4 * half : 4 * half + 4, sl * P : (sl + 1) * P],
                        start=True,
                        stop=True,
                        skip_group_check=True,
                    )
                pe_sb = ptpool.tile(
                    [P, HEADS * P], bf16, tag="pe", name=f"pe{sl}_{jt}"
                )
                nc.scalar.activation(pe_sb[:, :], sT[:, :], AF.Exp, scale=0.125)
                pt_sb = ptpool.tile(
                    [P, HEADS * P], bf16, tag="pt", name=f"pt{sl}_{jt}", bufs=5
                )
                # while a tail chain is draining, the pair multiply rides
                # GPSIMD (slower but otherwise idle) -- DVE is saturated by
                # the tail's copy/recip/normalize ops in those windows
                if tail_tasks:
                    gps_cool = 2
                eng = nc.gpsimd if gps_cool > 0 else nc.vector
                gps_cool = max(0, gps_cool - 1)
                mul_i += 1
                eng.tensor_mul(
                    pt_sb[:, :], pe_sb[:, :], bias_sb[:, bcol : bcol + HEADS * P]
                )
                pend_pv.append((sl, jt, pt_sb))
                while len(pend_pv) > pv_depth:
                    emit_pv(*pend_pv.pop(0))
                # keep-warm: bare LDWEIGHTS occupy the PE (~128 cycles each,
                # no PSUM writes) so HAM's activity window never sees the
                # ~0.2us/pair idle that ACT pacing would otherwise leave --
                # cold (K=4/8, 1.2GHz) matmuls cost far more than the filler
                for _ in range(2):
                    nc.tensor.ldweights(VST[:, 0:P])
                w = splice_kv.get((sl, jt))
                if w is not None:
                    emit_kv(w)
                w = splice_tp.get((sl, jt))
                if w is not None:
                    emit_tp(w)
                if tail_tasks:
                    tail_tasks.pop(0)()
            while pend_pv:
                emit_pv(*pend_pv.pop(0))
            while tail_tasks:
                tail_tasks.pop(0)()

    nc.compile()
    return nc


def prep_inputs(x, rel_pos_bias, Wq, Wkv, Wo, bo):
    """Build the 8 per-core input maps (host-side sharding/marshalling)."""
    x = np.asarray(x, dtype=np.float32)
    rel_pos_bias = np.asarray(rel_pos_bias, dtype=np.float32)
    Wq = np.ascontiguousarray(np.asarray(Wq, dtype=np.float32))
    Wkv = np.ascontiguousarray(np.asarray(Wkv, dtype=np.float32))
    Wo = np.ascontiguousarray(np.asarray(Wo, dtype=np.float32))
    bo = np.asarray(bo, dtype=np.float32).reshape(1, DIM)
    ident = np.concatenate([np.eye(DH), np.eye(DH)], axis=0).astype(BF16)

    exp_rpb = np.exp(rel_pos_bias)  # [h, n, n]
    tri = np.triu(np.ones((P, P), dtype=bool), k=1)  # intra-tile causal mask

    def chunked(w, nch):
        """[nch*128, cols] -> [128, nch*cols] with chunk-major free dim
        (the SBUF layout: partition p, chunk c at free offset c*cols)."""
        cols = w.shape[1]
        return np.ascontiguousarray(
            w.reshape(nch, P, cols).transpose(1, 0, 2).reshape(P, nch * cols)
        ).astype(BF16)

    # xTb per batch: [p, w, c, t] so each 512-token window is one
    # contiguous [128, 8*512] 2D DMA
    xTb8 = []
    for b in range(B):
        xT = x[b].T.reshape(8, P, NWIN, 512)  # [c, p, w, t]
        xTb8.append(
            np.ascontiguousarray(xT.transpose(1, 2, 0, 3).reshape(P, -1)).astype(BF16)
        )
    Wq8 = chunked(Wq, 8)
    Wkv8 = chunked(Wkv, 8)
    Wo8 = chunked(Wo, 4)
    ones_r = np.ones((1, P), np.float32)

    in_maps = []
    for c in range(NCORES):
        b = c // 4
        tiles = _q_tiles(c)
        xs = [x[b, t * P : (t + 1) * P, :] for t in tiles]
        xTo = chunked(np.concatenate(xs, axis=0).T, 8)

        npair = sum(t + 1 for t in tiles)  # exact causal pairs (34)
        biasT = np.empty((npair, P, HEADS, P), dtype=np.float32)
        base = 0
        for sl, t in enumerate(tiles):
            ext_r = t + 1  # exact causal extent
            # [h, q, j] block for all true j-tiles -> [jt, j, h, q]
            blk = exp_rpb[:, t * P : (t + 1) * P, : ext_r * P]
            blk = blk.reshape(HEADS, P, ext_r, P).transpose(2, 3, 0, 1).copy()
            # diagonal tile: zero strictly-future entries (j > q)
            blk[ext_r - 1] = np.where(tri.T[:, None, :], 0.0, blk[ext_r - 1])
            biasT[base : base + ext_r] = blk
            base += ext_r
        # -> [j, pair, h, q]: each bias-group tile is one contiguous 2D DMA
        biasT = np.ascontiguousarray(
            biasT.transpose(1, 0, 2, 3).reshape(P, npair * HEADS * P)
        )
        in_maps.append(
            {
                "xTb": xTb8[b],
                "xTo": xTo,
                "Wq": Wq8,
                "Wkv": Wkv8,
                "Wo": Wo8,
                "bo": bo,
                "ident": ident,
                "ones_r": ones_r,
                "biasT": biasT.astype(BF16),
            }
        )
    return in_maps


def assemble(outs):
    """outs: list of 8 [512, 1024] bf16 arrays -> full [2, 2048, 1024] f32."""
    full = np.empty((B, N, DIM), dtype=np.float32)
    for c in range(NCORES):
        b = c // 4
        o = np.asarray(outs[c]).astype(np.float32)
        for sl, t in enumerate(_q_tiles(c)):
            full[b, t * P : (t + 1) * P, :] = o[sl * P : (sl + 1) * P]
    return full


def _io_spec(nc):
    """(input names, output names, output avals, zero buffers) for nc."""
    import jax
    import concourse.mybir as mybir

    in_names, out_names, out_avals, zeros = [], [], [], []
    for alloc in nc.m.functions[0].allocations:
        if not isinstance(alloc, mybir.MemoryLocationSet):
            continue
        name = alloc.memorylocations[0].name
        if alloc.kind == "ExternalInput":
            in_names.append(name)
        elif alloc.kind == "ExternalOutput":
            shape = tuple(alloc.tensor_shape)
            dtype = mybir.dt.np(alloc.dtype)
            out_names.append(name)
            out_avals.append(jax.core.ShapedArray(shape, dtype))
            zeros.append(np.zeros(shape, dtype))
    return in_names, out_names, out_avals, zeros


def _dispatch_async(nc, in_map, device):
    """Launch nc's NEFF on `device` via PJRT without materializing outputs
    (mirrors bass2jax.run_bass_via_pjrt's single-core branch, minus the
    blocking np.asarray, so all 8 cores' launches overlap)."""
    import jax
    from concourse import bass2jax

    bass2jax.install_neuronx_cc_hook()
    assert nc.dbg_addr is None
    in_names, out_names, out_avals, zeros = _io_spec(nc)
    operands = [np.asarray(in_map[n]) for n in in_names]
    all_names = tuple(in_names + out_names)

    def _body(*args):
        return tuple(
            bass2jax._bass_exec_p.bind(
                *args,
                out_avals=tuple(out_avals),
                in_names=all_names,
                out_names=tuple(out_names),
                lowering_input_output_aliases=(),
                sim_require_finite=True,
                sim_require_nnan=True,
                nc=nc,
            )
        )

    donate = tuple(range(len(in_names), len(in_names) + len(out_names)))
    with jax.default_device(device):
        arrs = jax.jit(_body, donate_argnums=donate, keep_unused=True)(
            *operands, *zeros
        )
    return dict(zip(out_names, arrs))


def kernel(**inputs):
    import jax
    from concourse.bass_utils import run_bass_kernel_spmd

    if "ncs" not in _CACHE:
        _CACHE["ncs"] = [build_graph(cp) for cp in range(4)]
    ncs = _CACHE["ncs"]
    in_maps = prep_inputs(
        inputs["x"], inputs["rel_pos_bias"], inputs["Wq"], inputs["Wkv"],
        inputs["Wo"], inputs["bo"],
    )
    trace = bool(int(os.environ.get("KERNEL_TRACE", "0")))
    devices = jax.devices()[:NCORES]
    results = [None] * NCORES
    start = 0
    if trace:
        # profile one core's program through the full NTFF pipeline on
        # device 0 (KERNEL_TRACE_CORE selects which; default core 0)
        tc = int(os.environ.get("KERNEL_TRACE_CORE", "0"))
        res0 = run_bass_kernel_spmd(ncs[tc % 4], [in_maps[tc]], core_ids=[0], trace=True)
        _CACHE["last_results"] = res0
        if tc == 0:
            results[0] = res0.results[0]
            start = 1
    else:
        _CACHE["last_results"] = None
    lazy = []
    for c in range(start, NCORES):
        im = dict(in_maps[c])
        pt = ncs[c % 4].partition_id_tensor
        if pt is not None:
            im[pt.name] = np.array([[c]], dtype=np.uint32)
        lazy.append((c, _dispatch_async(ncs[c % 4], im, devices[c])))
    for c, m in lazy:
        results[c] = {k: np.asarray(v) for k, v in m.items()}
    return assemble([r["out"] for r in results])
